# revision 50
# baseline (speedup 1.0000x reference)
"""ConvSTFT on Trainium2: strided conv of x[32, 480000] against a fixed
[514, 1, 400] Fourier basis, hop 100 -> out [32, 514, 4803] f32.

Sharding: pure data parallel. Batch dim (32) split 4-per-core across 8
NeuronCores; the small weight is replicated.

Host prep (sharding layer): pad x by 300 on both sides, then lay it out
chunk-transposed in blocks of 128 hops:
    x_dev[b, r, s, p] = x_padded[b, (128 s + p) * 100 + r]
so the device can DMA straight into XT[r, f'] = x_padded[100 f' + r]
(f' = 128 s + p) with long (s-range * 256 B) contiguous lines. The
weight is passed ctile-major [ct, r, j, 128] (channels zero-padded to a
multiple of 128) so per-ctile weight DMAs read contiguous 1KB partition
lines. Both bf16.

Channel packing: the Fourier basis has two identically-zero rows — the
imaginary parts of bin 0 and of the Nyquist bin (channels 257 and 513).
The host drops them, leaving exactly 512 = 4x128 channels; the device
computes and stores a packed [512, n_frames] output and the host
re-inserts the two zero rows while gathering. This removes the 5th
channel tile (which carried only 2 live rows but cost a full N-column
stream), cutting PE work by 20%. If a future weight doesn't have the
zero rows, a generic 514-channel (5-tile) program is compiled instead.

Per-core device kernel (Bass/Tile):
  t = 100j + r decomposition (j in 0..3, r in 0..99) turns the overlapped
  conv into 4 PSUM-accumulated matmuls:
      out[c, f] = sum_j sum_r wt[100j + r, c] * XT[r, f + j]
  - lhsT = wt[r, j, c-tile] (K=100, M=128), rhs = XT[r, f-tile] (N<=512),
    fp32 PSUM accumulation over j, all 8 PSUM banks in flight.
  - PSUM is evacuated by DVE only (~690ns per 128x512 group vs the 852ns
    group cadence), converting to bf16 into an SBUF row [128, 4803].
    Keeping ACTIVATE off the scalar engine avoids the act-table preamble
    load that would delay the ACT queue's first DMA dispatch by ~1.2us.
  - bf16 output stores (the host upconverts to f32; adds <=2^-9 relative
    error vs the 2e-2 budget): the f32 store stream (~290 GB/s demand)
    exceeded what the two DMA queues deliver (~160-185 GB/s each, warm)
    and left a ~14us drain tail; bf16 halves it.
  - Each (batch, ctile) row is stored in 3 pieces as its copies land
    (4 for the final row, whose last piece is only 195 cols so almost
    nothing drains after the final matmul), alternating across the SP
    and ACT queues.
  - DMA scheduling is the main subtlety: only SP and ACT have hardware
    DGE queues, each ramping ~50->185 GB/s with sustained activity and
    descriptor size. The critical first loads (ctile-0 weights on SP,
    xt blocks 0-5 on ACT) land ~10us; following xt pieces are split in
    consumption order across both queues so each lands just before the
    stream needs it; the three 0.95MB whole-batch loads are pinned past
    the critical window with tile_wait_until (the Tile scheduler would
    otherwise hoist them to the front, where they measurably starve the
    critical pieces and stall the stream ~5us).
  - Startup: 7 warmup matmuls bridge the PE from ~7.8us (after the fixed
    ~6.6us framework engine-barrier preamble) to the first real matmul
    at ~10.8us, so the HAM clock gate (lifts to 2.4 GHz after ~3.4us of
    CONTINUOUS PE activity; a gap resets the ramp and costs ~7us in
    re-throttle) is already open when the real stream starts.
This streams the PE at its floor (1 bf16 column/cycle; 16 tile-streams
per frame-column = ceil(512/128) * ceil(400/128) is minimal at this
dtype — fp8 DoubleRow needs a 3-term hi/lo split to meet the 2e-2
tolerance and nets 0.59x, and FFT factorizations of the basis need 4-5x
input traffic, both net losses). Measured ~147.5 us on a cool chip
(~6.6 preamble + ~4.2 warmup/load bridge + ~131.4 matmul stream at the
2.4 GHz floor + ~5 drain/epilogue); chip-level power management (P0,
PE at 2.0 GHz) adds up to ~20% when hot.
"""

import numpy as np
import ml_dtypes

WIN, HOP, C = 400, 100, 514
B, T = 32, 480000
PAD = WIN - HOP                       # 300
N_CORES = 8
B_LOC = B // N_CORES                  # 4
T_PAD = T + 2 * PAD                   # 480600
N_FRAMES = (T_PAD - WIN) // HOP + 1   # 4803
S_BLOCKS = -(-(T_PAD // HOP) // 128)  # 38 blocks of 128 chunks
N_CHUNKS = S_BLOCKS * 128             # 4864
NJ = WIN // HOP                       # 4

F_TILE = 512
C_TILE = 128
STORE_SPLIT = 5                       # anchor ftile index for store cuts
ZCH = (257, 513)                      # identically-zero basis channels


def build_program(packed, b_loc=B_LOC, s_blocks=S_BLOCKS, n_frames=N_FRAMES):
    import concourse.bacc as bacc
    import concourse.mybir as mybir
    import concourse.tile as tile

    dt = mybir.dt
    n_chunks = s_blocks * 128
    assert n_frames + NJ - 1 <= n_chunks
    cp = 512 if packed else C

    nc = bacc.Bacc("TRN2", target_bir_lowering=False, debug=False)
    x_d = nc.dram_tensor(
        "x", [b_loc, HOP, s_blocks, 128], dt.bfloat16, kind="ExternalInput"
    ).ap()
    # weight arrives host-prearranged ctile-major [ct, r, j, 128] (channel
    # dim zero-padded to a multiple of 128) so each per-ctile weight DMA
    # reads one contiguous 1KB line per partition — the first ctile's
    # weights are the critical path for the first real matmul
    n_ct = -(-cp // C_TILE)
    w_d = nc.dram_tensor(
        "wt", [n_ct, HOP, NJ, C_TILE], dt.bfloat16, kind="ExternalInput"
    ).ap()
    # bf16 output: halves the store stream (f32 stores average ~280 GB/s over
    # the matmul stream, right at the ring ceiling, leaving a ~14 us drain
    # tail); the host upconverts to f32 (adds <=2^-9 relative, vs 2e-2 budget)
    o_d = nc.dram_tensor(
        "out", [b_loc, cp, n_frames], dt.bfloat16, kind="ExternalOutput"
    ).ap()

    ctiles = [(c0, min(C_TILE, cp - c0)) for c0 in range(0, cp, C_TILE)]
    ftiles = [(f0, min(F_TILE, n_frames - f0)) for f0 in range(0, n_frames, F_TILE)]

    n_ft = len(ftiles)

    # store each (batch, ctile) row in 3 pieces as its copies land: smooths
    # store traffic (~145 GB/s aggregate demand vs ~160-185 GB/s per warm
    # queue) and keeps partition lines >= 1.4KB. The very last row instead
    # splits its tail (the last 707 cols) into four small matmul groups
    # whose pieces store progressively, so only a ~24KB piece remains to
    # drain after the final matmul (the drain chain — evac, trigger,
    # descriptor fetch, transfer — was ~3.2us when gated by a 131KB piece).
    def row_plan(i):
        fts = list(ftiles)
        idx = (STORE_SPLIT - 1, STORE_SPLIT + 2, n_ft - 1)
        if i == b_loc * len(ctiles) - 1 and n_ft >= 2:
            f0, fn = fts[-2]
            tail = fts[-2][1] + fts[-1][1]
            fts = fts[:-2]
            for step in (256, 256, (tail - 512 + 1) // 2, 0):
                fn2 = min(step, tail) if step else tail
                if fn2 <= 0:
                    break
                fts.append((f0, fn2))
                f0 += fn2
                tail -= fn2
            idx = (STORE_SPLIT - 1, STORE_SPLIT + 2) + tuple(
                range(n_ft - 2, len(fts))
            )
        cuts = sorted(
            {fts[min(j, len(fts) - 1)][0] + fts[min(j, len(fts) - 1)][1]
             for j in idx}
        )
        return fts, cuts

    with tile.TileContext(nc) as tc:
        with (
            tc.tile_pool(name="const", bufs=1) as constp,
            tc.tile_pool(name="xt", bufs=4) as xtp,
            tc.tile_pool(name="orow", bufs=7) as orowp,
            tc.tile_pool(name="mmps", bufs=8, space="PSUM") as mmps,
        ):
            # Warm the PE clock gate (HAM) with throwaway matmuls while the
            # first input DMAs are in flight (the clock lifts from 1.2 to
            # 2.4 GHz after ~3.4us of CONTINUOUS PE activity; these bridge
            # the PE until the first weights/rhs land ~10.8us). Two cold
            # DMA queues (~90-110 GB/s until their activity ramp lifts
            # them) physically cannot deliver the first ~660KB of
            # weights+rhs any sooner, and an earlier stream start just
            # trades warmup for stalls — which also re-throttle the HAM
            # clock (a ~7us penalty measured).
            warm = constp.tile([128, 512], dt.bfloat16)
            nc.gpsimd.memset(warm[:], 0.0)
            wps = mmps.tile([128, F_TILE], dt.float32, tag="ps")
            for _ in range(7):
                nc.tensor.matmul(wps[0:16, :], warm[:, 0:16], warm[:])

            # Critical first loads, interleaved across the two hardware DMA
            # queues (only the SP and ACT engines have HW DGE queues; each
            # runs ~100-175 GB/s depending on descriptor size). The first
            # ctile's weights and the first xt blocks go first — one on each
            # queue — then xt pieces sized so each lands just before the
            # stream consumes it (~150 GB/s aggregate needed for the first
            # 4 ftiles), then the remaining weights and whole batches. ALL
            # loads are dispatched up front (xt pool holds all 4 batches),
            # so no load trigger ever queues behind a semaphore-gated store
            # trigger; that in turn makes it safe to spread stores across
            # both rings. No scalar-engine ACTIVATE is used anywhere (evac
            # is DVE-only) so no ACT_TABLE_LOAD delays the ACT queue's
            # first dispatch.
            wsb = constp.tile([HOP, n_ct, NJ, C_TILE], dt.bfloat16)
            xts = [
                xtp.tile(
                    [HOP, s_blocks, 128], dt.bfloat16, tag="xt", name=f"xt{b}"
                )
                for b in range(b_loc)
            ]
            xt0 = xts[0]

            def ld_w(eng, k):
                eng.dma_start(wsb[:, k, :, :], w_d[k], single_packet=True)

            def ld_x(eng, b, g0, g1):
                eng.dma_start(
                    xts[b][:, g0:g1, :], x_d[b, :, g0:g1, :],
                    single_packet=True,
                )

            # xt0 pieces split across both queues in consumption order
            # (ftile k of ctile 0 consumes ~5 blocks every 852ns — ~150 GB/s
            # aggregate for the first 4 ftiles, beyond one cold queue's
            # rate); the later weights ride behind (w-ctile k is first
            # needed at ~+8.5k us). Batch b=1..3 loads are NOT dispatched
            # here: they'd hog the queues during this critical window (each
            # is 0.95MB); they're emitted after batch-0 ctile rows below.
            sb = s_blocks
            ld_w(nc.sync, 0)                       # q1: ctile-0 weights
            ld_x(nc.scalar, 0, 0, min(5, sb))      # q10: ftile-0 rhs
            for g0, g1 in [(5, 9), (9, 13), (13, 17)]:
                if sb > g0:
                    ld_x(nc.sync, 0, g0, min(g1, sb))
            for g0, g1 in [(17, 22), (22, 27), (27, 32), (32, sb)]:
                if sb > g0:
                    ld_x(nc.scalar, 0, g0, min(g1, sb))
            for k in range(1, n_ct):
                ld_w(nc.sync, k)       # first needed when ctile 1 starts

            def mm_group(xtf, orow, b, k, c0, cm, f0, fn):
                ps = mmps.tile([128, F_TILE], dt.float32, tag="ps")
                for j in range(NJ):
                    nc.tensor.matmul(
                        ps[0:cm, 0:fn],
                        wsb[0:HOP, k, j, 0:cm],
                        xtf[0:HOP, f0 + j : f0 + j + fn],
                        start=(j == 0),
                        stop=(j == NJ - 1),
                    )
                # DVE-only evacuation: ~690ns per group vs the 852ns group
                # cadence, with 8 PSUM banks of slack; keeping ACTIVATE off
                # the scalar engine avoids the act-table preamble load
                nc.vector.tensor_copy(orow[0:cm, f0 : f0 + fn], ps[0:cm, 0:fn])

            npiece = 0
            for b in range(b_loc):
                xtf = xts[b].rearrange("r g p -> r (g p)")

                for k, (c0, cm) in enumerate(ctiles):
                    fts, cuts = row_plan(b * len(ctiles) + k)
                    orow = orowp.tile([128, n_frames], dt.bfloat16, tag="orow")
                    prev = 0
                    for f0, fn in fts:
                        mm_group(xtf, orow, b, k, c0, cm, f0, fn)
                        if f0 + fn in cuts:
                            # alternate pieces across the SP and ACT rings:
                            # each ring runs well under half its burst
                            # bandwidth, so no store backlog survives the
                            # end of the matmul stream
                            eng = nc.scalar if npiece % 2 == 1 else nc.sync
                            eng.dma_start(
                                o_d[b, c0 : c0 + cm, prev : f0 + fn],
                                orow[0:cm, prev : f0 + fn],
                            )
                            prev = f0 + fn
                            npiece += 1
                    if b == 0 and k + 1 < b_loc:
                        # whole-batch load for batch k+1 (9.7KB partition
                        # lines run the warm queue at ~160-185 GB/s). The
                        # tile scheduler hoists dependency-free DMA
                        # triggers to the program start, where this 0.95MB
                        # transfer would starve the critical early xt/w
                        # loads (measured: x[17:22] crawled at 35 GB/s and
                        # stalled the stream 4.6us); tile_wait_until pins
                        # it past the critical window. Batch k+1's matmuls
                        # start ~(k+1)*34us in.
                        eng = nc.sync if k == 1 else nc.scalar
                        with tc.tile_wait_until(0.018 + 0.012 * k):
                            eng.dma_start(
                                xts[k + 1][:], x_d[k + 1], single_packet=True
                            )

    nc.compile()
    return nc


_NC = {}
LAST_RESULTS = None


def _ensure_axon_hooks_stub():
    """If BASS_TRACE is set but the container's antenv lacks axon_hooks,
    run_bass_kernel_spmd would crash on import; degrade to no-trace."""
    import sys

    try:
        import antenv.axon_hooks  # noqa: F401
    except ImportError:
        import types

        import antenv

        m = types.ModuleType("antenv.axon_hooks")
        m.get_axon_ntff_profile_hook = lambda: None
        m.set_axon_ntff_profile_hook = lambda h: None
        sys.modules["antenv.axon_hooks"] = m
        antenv.axon_hooks = m


def _prep_inputs(x, weight):
    x = np.asarray(x, dtype=np.float32)
    w = np.asarray(weight, dtype=np.float32)
    nb = x.shape[0]
    xp = np.zeros((nb, N_CHUNKS * HOP), dtype=np.float32)
    xp[:, PAD : PAD + x.shape[1]] = x
    # chunk-block transpose: [b, s, p, r] -> [b, r, s, p] so that device
    # loads of xt[r, s-range, p] read gs*256-byte contiguous DRAM lines
    xdev = np.ascontiguousarray(
        xp.reshape(nb, S_BLOCKS, 128, HOP).transpose(0, 3, 1, 2)
    ).astype(ml_dtypes.bfloat16)
    w2 = w.reshape(C, WIN)
    scale = np.abs(w2).max()
    packed = all(np.abs(w2[z]).max() <= 1e-6 * scale for z in ZCH)
    if packed:
        w2 = np.delete(w2, list(ZCH), axis=0)
    # ctile-major [ct, r, j, 128] layout (channels zero-padded to a multiple
    # of 128): each per-ctile weight DMA reads contiguous 1KB partition lines
    cp = w2.shape[0]
    n_ct = -(-cp // 128)
    wp = np.zeros((n_ct * 128, NJ, HOP), dtype=np.float32)
    wp[:cp] = w2.reshape(cp, NJ, HOP)
    wt = np.ascontiguousarray(
        wp.reshape(n_ct, 128, NJ, HOP).transpose(0, 3, 2, 1)
    ).astype(ml_dtypes.bfloat16)
    return xdev, wt, packed


def kernel(x, weight):
    global LAST_RESULTS
    from concourse.bass_utils import run_bass_kernel_spmd

    _ensure_axon_hooks_stub()
    xdev, wt, packed = _prep_inputs(x, weight)
    if packed not in _NC:
        _NC[packed] = build_program(packed)
    in_maps = [
        {"x": np.ascontiguousarray(xdev[c * B_LOC : (c + 1) * B_LOC]), "wt": wt}
        for c in range(N_CORES)
    ]
    res = run_bass_kernel_spmd(_NC[packed], in_maps, core_ids=list(range(N_CORES)))
    LAST_RESULTS = res
    # device stores bf16; upconvert to f32 while gathering
    outp = np.concatenate(
        [np.asarray(r["out"]).astype(np.float32) for r in res.results], axis=0
    )
    if not packed:
        return np.ascontiguousarray(outp)
    # re-insert the two zero channels dropped on device
    out = np.empty((outp.shape[0], C, outp.shape[2]), dtype=np.float32)
    out[:, 0:257] = outp[:, 0:257]
    out[:, 257] = 0.0
    out[:, 258:513] = outp[:, 257:512]
    out[:, 513] = 0.0
    return out



# revision 51
# speedup vs baseline: 1.0056x; 1.0056x over previous
"""ConvSTFT on Trainium2: strided conv of x[32, 480000] against a fixed
[514, 1, 400] Fourier basis, hop 100 -> out [32, 514, 4803] f32.

Sharding: pure data parallel. Batch dim (32) split 4-per-core across 8
NeuronCores; the small weight is replicated.

Host prep (sharding layer): pad x by 300 on both sides, then lay it out
chunk-transposed in blocks of 128 hops:
    x_dev[b, r, s, p] = x_padded[b, (128 s + p) * 100 + r]
so the device can DMA straight into XT[r, f'] = x_padded[100 f' + r]
(f' = 128 s + p) with long (s-range * 256 B) contiguous lines. The
weight is passed ctile-major [ct, r, j, 128] (channels zero-padded to a
multiple of 128) so per-ctile weight DMAs read contiguous 1KB partition
lines. Both bf16.

Channel packing: the Fourier basis has two identically-zero rows — the
imaginary parts of bin 0 and of the Nyquist bin (channels 257 and 513).
The host drops them, leaving exactly 512 = 4x128 channels; the device
computes and stores a packed [512, n_frames] output and the host
re-inserts the two zero rows while gathering. This removes the 5th
channel tile (which carried only 2 live rows but cost a full N-column
stream), cutting PE work by 20%. If a future weight doesn't have the
zero rows, a generic 514-channel (5-tile) program is compiled instead.

Per-core device kernel (Bass/Tile):
  t = 100j + r decomposition (j in 0..3, r in 0..99) turns the overlapped
  conv into 4 PSUM-accumulated matmuls:
      out[c, f] = sum_j sum_r wt[100j + r, c] * XT[r, f + j]
  - lhsT = wt[r, j, c-tile] (K=100, M=128), rhs = XT[r, f-tile] (N<=512),
    fp32 PSUM accumulation over j, all 8 PSUM banks in flight.
  - PSUM is evacuated by DVE only (~690ns per 128x512 group vs the 852ns
    group cadence), converting to bf16 into an SBUF row [128, 4803].
    Keeping ACTIVATE off the scalar engine avoids the act-table preamble
    load that would delay the ACT queue's first DMA dispatch by ~1.2us.
  - bf16 output stores (the host upconverts to f32; adds <=2^-9 relative
    error vs the 2e-2 budget): the f32 store stream (~290 GB/s demand)
    exceeded what the two DMA queues deliver (~160-185 GB/s each, warm)
    and left a ~14us drain tail; bf16 halves it.
  - Each (batch, ctile) row is stored in 3 pieces as its copies land
    (4 for the final row, whose last piece is only 195 cols so almost
    nothing drains after the final matmul), alternating across the SP
    and ACT queues.
  - DMA scheduling is the main subtlety: only SP and ACT have hardware
    DGE queues, each ramping ~50->185 GB/s with sustained activity and
    descriptor size. The critical first loads (ctile-0 weights on SP,
    xt blocks 0-5 on ACT) land ~10us; following xt pieces are split in
    consumption order across both queues so each lands just before the
    stream needs it; the three 0.95MB whole-batch loads are pinned past
    the critical window with tile_wait_until (the Tile scheduler would
    otherwise hoist them to the front, where they measurably starve the
    critical pieces and stall the stream ~5us).
  - Startup: 7 warmup matmuls bridge the PE from ~7.8us (after the fixed
    ~6.6us framework engine-barrier preamble) to the first real matmul
    at ~10.8us, so the HAM clock gate (lifts to 2.4 GHz after ~3.4us of
    CONTINUOUS PE activity; a gap resets the ramp and costs ~7us in
    re-throttle) is already open when the real stream starts.
This streams the PE at its floor (1 bf16 column/cycle; 16 tile-streams
per frame-column = ceil(512/128) * ceil(400/128) is minimal at this
dtype — fp8 DoubleRow needs a 3-term hi/lo split to meet the 2e-2
tolerance and nets 0.59x, and FFT factorizations of the basis need 4-5x
input traffic, both net losses). Measured ~147.5 us on a cool chip
(~6.6 preamble + ~4.2 warmup/load bridge + ~131.4 matmul stream at the
2.4 GHz floor + ~5 drain/epilogue); chip-level power management (P0,
PE at 2.0 GHz) adds up to ~20% when hot.
"""

import numpy as np
import ml_dtypes

WIN, HOP, C = 400, 100, 514
B, T = 32, 480000
PAD = WIN - HOP                       # 300
N_CORES = 8
B_LOC = B // N_CORES                  # 4
T_PAD = T + 2 * PAD                   # 480600
N_FRAMES = (T_PAD - WIN) // HOP + 1   # 4803
S_BLOCKS = -(-(T_PAD // HOP) // 128)  # 38 blocks of 128 chunks
N_CHUNKS = S_BLOCKS * 128             # 4864
NJ = WIN // HOP                       # 4

F_TILE = 512
C_TILE = 128
STORE_SPLIT = 5                       # anchor ftile index for store cuts
ZCH = (257, 513)                      # identically-zero basis channels


def build_program(packed, b_loc=B_LOC, s_blocks=S_BLOCKS, n_frames=N_FRAMES):
    import concourse.bacc as bacc
    import concourse.mybir as mybir
    import concourse.tile as tile

    dt = mybir.dt
    n_chunks = s_blocks * 128
    assert n_frames + NJ - 1 <= n_chunks
    cp = 512 if packed else C

    nc = bacc.Bacc("TRN2", target_bir_lowering=False, debug=False)
    x_d = nc.dram_tensor(
        "x", [b_loc, HOP, s_blocks, 128], dt.bfloat16, kind="ExternalInput"
    ).ap()
    # weight arrives host-prearranged ctile-major [ct, r, j, 128] (channel
    # dim zero-padded to a multiple of 128) so each per-ctile weight DMA
    # reads one contiguous 1KB line per partition — the first ctile's
    # weights are the critical path for the first real matmul
    n_ct = -(-cp // C_TILE)
    w_d = nc.dram_tensor(
        "wt", [n_ct, HOP, NJ, C_TILE], dt.bfloat16, kind="ExternalInput"
    ).ap()
    # bf16 output: halves the store stream (f32 stores average ~280 GB/s over
    # the matmul stream, right at the ring ceiling, leaving a ~14 us drain
    # tail); the host upconverts to f32 (adds <=2^-9 relative, vs 2e-2 budget)
    o_d = nc.dram_tensor(
        "out", [b_loc, cp, n_frames], dt.bfloat16, kind="ExternalOutput"
    ).ap()

    ctiles = [(c0, min(C_TILE, cp - c0)) for c0 in range(0, cp, C_TILE)]
    ftiles = [(f0, min(F_TILE, n_frames - f0)) for f0 in range(0, n_frames, F_TILE)]

    n_ft = len(ftiles)

    # store each (batch, ctile) row in 3 pieces as its copies land: smooths
    # store traffic (~145 GB/s aggregate demand vs ~160-185 GB/s per warm
    # queue) and keeps partition lines >= 1.4KB. The very last row gets a
    # finer split so only a small 195-col piece drains after the final
    # matmul. (A finer tail split was measured WORSE: the drain is
    # dominated by per-piece fixed costs — trigger ~0.6us, descriptor
    # fetch ~0.7us, semaphore — not by transfer size.)
    def row_plan(i):
        idx = (STORE_SPLIT - 1, STORE_SPLIT + 2, n_ft - 1)
        if i == b_loc * len(ctiles) - 1:
            idx = (STORE_SPLIT - 1, STORE_SPLIT + 2, n_ft - 2, n_ft - 1)
        cuts = sorted(
            {ftiles[min(j, n_ft - 1)][0] + ftiles[min(j, n_ft - 1)][1]
             for j in idx}
        )
        return list(ftiles), cuts

    with tile.TileContext(nc) as tc:
        with (
            tc.tile_pool(name="const", bufs=1) as constp,
            tc.tile_pool(name="xt", bufs=4) as xtp,
            tc.tile_pool(name="orow", bufs=7) as orowp,
            tc.tile_pool(name="mmps", bufs=8, space="PSUM") as mmps,
        ):
            # Warm the PE clock gate (HAM) with throwaway matmuls while the
            # first input DMAs are in flight (the clock lifts from 1.2 to
            # 2.4 GHz after ~3.4us of CONTINUOUS PE activity; these bridge
            # the PE until the first weights/rhs land ~10.8us). Two cold
            # DMA queues (~90-110 GB/s until their activity ramp lifts
            # them) physically cannot deliver the first ~660KB of
            # weights+rhs any sooner, and an earlier stream start just
            # trades warmup for stalls — which also re-throttle the HAM
            # clock (a ~7us penalty measured).
            warm = constp.tile([128, 512], dt.bfloat16)
            nc.gpsimd.memset(warm[:], 0.0)
            wps = mmps.tile([128, F_TILE], dt.float32, tag="ps")
            for _ in range(7):
                nc.tensor.matmul(wps[0:16, :], warm[:, 0:16], warm[:])

            # Critical first loads, interleaved across the two hardware DMA
            # queues (only the SP and ACT engines have HW DGE queues; each
            # runs ~100-175 GB/s depending on descriptor size). The first
            # ctile's weights and the first xt blocks go first — one on each
            # queue — then xt pieces sized so each lands just before the
            # stream consumes it (~150 GB/s aggregate needed for the first
            # 4 ftiles), then the remaining weights and whole batches. ALL
            # loads are dispatched up front (xt pool holds all 4 batches),
            # so no load trigger ever queues behind a semaphore-gated store
            # trigger; that in turn makes it safe to spread stores across
            # both rings. No scalar-engine ACTIVATE is used anywhere (evac
            # is DVE-only) so no ACT_TABLE_LOAD delays the ACT queue's
            # first dispatch.
            wsb = constp.tile([HOP, n_ct, NJ, C_TILE], dt.bfloat16)
            xts = [
                xtp.tile(
                    [HOP, s_blocks, 128], dt.bfloat16, tag="xt", name=f"xt{b}"
                )
                for b in range(b_loc)
            ]
            xt0 = xts[0]

            def ld_w(eng, k):
                eng.dma_start(wsb[:, k, :, :], w_d[k], single_packet=True)

            def ld_x(eng, b, g0, g1):
                eng.dma_start(
                    xts[b][:, g0:g1, :], x_d[b, :, g0:g1, :],
                    single_packet=True,
                )

            # xt0 pieces split across both queues in consumption order
            # (ftile k of ctile 0 consumes ~5 blocks every 852ns — ~150 GB/s
            # aggregate for the first 4 ftiles, beyond one cold queue's
            # rate); the later weights ride behind (w-ctile k is first
            # needed at ~+8.5k us). Batch b=1..3 loads are NOT dispatched
            # here: they'd hog the queues during this critical window (each
            # is 0.95MB); they're emitted after batch-0 ctile rows below.
            sb = s_blocks
            ld_w(nc.sync, 0)                       # q1: ctile-0 weights
            ld_x(nc.scalar, 0, 0, min(5, sb))      # q10: ftile-0 rhs
            for g0, g1 in [(5, 9), (9, 13), (13, 17)]:
                if sb > g0:
                    ld_x(nc.sync, 0, g0, min(g1, sb))
            for g0, g1 in [(17, 22), (22, 27), (27, 32), (32, sb)]:
                if sb > g0:
                    ld_x(nc.scalar, 0, g0, min(g1, sb))
            for k in range(1, n_ct):
                ld_w(nc.sync, k)       # first needed when ctile 1 starts

            def mm_group(xtf, orow, b, k, c0, cm, f0, fn):
                ps = mmps.tile([128, F_TILE], dt.float32, tag="ps")
                for j in range(NJ):
                    nc.tensor.matmul(
                        ps[0:cm, 0:fn],
                        wsb[0:HOP, k, j, 0:cm],
                        xtf[0:HOP, f0 + j : f0 + j + fn],
                        start=(j == 0),
                        stop=(j == NJ - 1),
                    )
                # DVE-only evacuation: ~690ns per group vs the 852ns group
                # cadence, with 8 PSUM banks of slack; keeping ACTIVATE off
                # the scalar engine avoids the act-table preamble load
                nc.vector.tensor_copy(orow[0:cm, f0 : f0 + fn], ps[0:cm, 0:fn])

            npiece = 0
            for b in range(b_loc):
                xtf = xts[b].rearrange("r g p -> r (g p)")

                for k, (c0, cm) in enumerate(ctiles):
                    fts, cuts = row_plan(b * len(ctiles) + k)
                    orow = orowp.tile([128, n_frames], dt.bfloat16, tag="orow")
                    prev = 0
                    for f0, fn in fts:
                        mm_group(xtf, orow, b, k, c0, cm, f0, fn)
                        if f0 + fn in cuts:
                            # alternate pieces across the SP and ACT rings:
                            # each ring runs well under half its burst
                            # bandwidth, so no store backlog survives the
                            # end of the matmul stream
                            eng = nc.scalar if npiece % 2 == 1 else nc.sync
                            eng.dma_start(
                                o_d[b, c0 : c0 + cm, prev : f0 + fn],
                                orow[0:cm, prev : f0 + fn],
                            )
                            prev = f0 + fn
                            npiece += 1
                    if b == 0 and k + 1 < b_loc:
                        # whole-batch load for batch k+1 (9.7KB partition
                        # lines run the warm queue at ~160-185 GB/s). The
                        # tile scheduler hoists dependency-free DMA
                        # triggers to the program start, where this 0.95MB
                        # transfer would starve the critical early xt/w
                        # loads (measured: x[17:22] crawled at 35 GB/s and
                        # stalled the stream 4.6us); tile_wait_until pins
                        # it past the critical window. Batch k+1's matmuls
                        # start ~(k+1)*34us in.
                        eng = nc.sync if k == 1 else nc.scalar
                        with tc.tile_wait_until(0.018 + 0.012 * k):
                            eng.dma_start(
                                xts[k + 1][:], x_d[k + 1], single_packet=True
                            )

    nc.compile()
    return nc


_NC = {}
LAST_RESULTS = None


def _ensure_axon_hooks_stub():
    """If BASS_TRACE is set but the container's antenv lacks axon_hooks,
    run_bass_kernel_spmd would crash on import; degrade to no-trace."""
    import sys

    try:
        import antenv.axon_hooks  # noqa: F401
    except ImportError:
        import types

        import antenv

        m = types.ModuleType("antenv.axon_hooks")
        m.get_axon_ntff_profile_hook = lambda: None
        m.set_axon_ntff_profile_hook = lambda h: None
        sys.modules["antenv.axon_hooks"] = m
        antenv.axon_hooks = m


def _prep_inputs(x, weight):
    x = np.asarray(x, dtype=np.float32)
    w = np.asarray(weight, dtype=np.float32)
    nb = x.shape[0]
    xp = np.zeros((nb, N_CHUNKS * HOP), dtype=np.float32)
    xp[:, PAD : PAD + x.shape[1]] = x
    # chunk-block transpose: [b, s, p, r] -> [b, r, s, p] so that device
    # loads of xt[r, s-range, p] read gs*256-byte contiguous DRAM lines
    xdev = np.ascontiguousarray(
        xp.reshape(nb, S_BLOCKS, 128, HOP).transpose(0, 3, 1, 2)
    ).astype(ml_dtypes.bfloat16)
    w2 = w.reshape(C, WIN)
    scale = np.abs(w2).max()
    packed = all(np.abs(w2[z]).max() <= 1e-6 * scale for z in ZCH)
    if packed:
        w2 = np.delete(w2, list(ZCH), axis=0)
    # ctile-major [ct, r, j, 128] layout (channels zero-padded to a multiple
    # of 128): each per-ctile weight DMA reads contiguous 1KB partition lines
    cp = w2.shape[0]
    n_ct = -(-cp // 128)
    wp = np.zeros((n_ct * 128, NJ, HOP), dtype=np.float32)
    wp[:cp] = w2.reshape(cp, NJ, HOP)
    wt = np.ascontiguousarray(
        wp.reshape(n_ct, 128, NJ, HOP).transpose(0, 3, 2, 1)
    ).astype(ml_dtypes.bfloat16)
    return xdev, wt, packed


def kernel(x, weight):
    global LAST_RESULTS
    from concourse.bass_utils import run_bass_kernel_spmd

    _ensure_axon_hooks_stub()
    xdev, wt, packed = _prep_inputs(x, weight)
    if packed not in _NC:
        _NC[packed] = build_program(packed)
    in_maps = [
        {"x": np.ascontiguousarray(xdev[c * B_LOC : (c + 1) * B_LOC]), "wt": wt}
        for c in range(N_CORES)
    ]
    res = run_bass_kernel_spmd(_NC[packed], in_maps, core_ids=list(range(N_CORES)))
    LAST_RESULTS = res
    # device stores bf16; upconvert to f32 while gathering
    outp = np.concatenate(
        [np.asarray(r["out"]).astype(np.float32) for r in res.results], axis=0
    )
    if not packed:
        return np.ascontiguousarray(outp)
    # re-insert the two zero channels dropped on device
    out = np.empty((outp.shape[0], C, outp.shape[2]), dtype=np.float32)
    out[:, 0:257] = outp[:, 0:257]
    out[:, 257] = 0.0
    out[:, 258:513] = outp[:, 257:512]
    out[:, 513] = 0.0
    return out

